# revision 2
# baseline (speedup 1.0000x reference)
"""3x3 same-conv (NHWC, 32x56x56x128 -> 32x56x56x256) + bias + ReLU on 8 TRN2 cores.

Strategy: data-parallel over batch (4 images/core). Per core, the conv is
9 shifted matmuls accumulated in PSUM with Cin=128 as the contraction dim:
  out[q, cout] = relu( sum_tap XpT[:, q+off_tap]^T @ W[tap] + b )
where XpT is the zero-padded image held transposed in SBUF ([cin, 58*58]
flat padded pixels, fp16). The anchor axis is tiled in 26 sliding windows
of exactly 128 contiguous padded positions per image (vs 28 row-pair
groups of 116): every matmul uses the full 128 PSUM partitions and the
128-column stationary operand enables fast weight load. Junk anchors
(pad columns) are computed but skipped at store time by splitting each
window into <=3 contiguous valid runs, each stored with its own HWDGE DMA
(contiguous [len,256] fp32 in DRAM).

The input transposes run on the DMA transpose XBAR (HWDGE/SP queue) into
a small [128,112] tmp, then an ACT-engine copy scatters the two 56-pixel
rows into the padded slab. The PE therefore runs nothing but conv
matmuls; a short warm-up accumulation at t=0 releases the HAM clock gate
before the first real matmul. Input stage loads are casting SWDGE DMAs
(fp32->fp16), all issued up front; output stores go on the ACT HWDGE
queue so the SWDGE drain happens mid-kernel, not in the tail.
"""

import os
from contextlib import ExitStack

import numpy as np

import concourse.bass as bass
import concourse.bacc as bacc
import concourse.mybir as mybir
import concourse.tile as tile
from concourse.bass_utils import run_bass_kernel_spmd

N_CORES = 8
B, H, W, CIN, COUT = 32, 56, 56, 128, 256
BPC = B // N_CORES            # images per core
S = W + 2                     # padded width (58)
PIMG = S * S                  # padded pixels per image (3364)
NW = 26                       # 128-anchor windows per image
WM = 128                      # anchors per window
SLAB_W = PIMG + 128           # per-image slab width incl. zero slop
RPC = 2                       # image rows per transpose chunk
CHUNK_PIX = RPC * W           # 112
NCHUNK = H // RPC             # 28

TAP_OFFS = [(dh - 1) * S + (dw - 1) for dh in range(3) for dw in range(3)]
F32 = mybir.dt.float32
F16 = mybir.dt.float16

LAST_RESULTS = None


def _window_runs(q0):
    """Contiguous valid-anchor runs inside window [q0, q0+WM).

    Returns (partition_offset, flat_output_pixel, length) triples; anchors
    at padded col 0/57 or pad rows are junk and fall between runs."""
    runs = []
    for r in range(q0 // S, (q0 + WM - 1) // S + 1):
        if r < 1 or r > H:
            continue
        lo = max(q0, r * S + 1)
        hi = min(q0 + WM, r * S + 1 + W)
        if lo < hi:
            runs.append((lo - q0, (r - 1) * W + (lo - r * S - 1), hi - lo))
    return runs


def _build(with_bias: bool):
    nc = bacc.Bacc("TRN2", target_bir_lowering=False, debug=False)
    x_h = nc.declare_dram_parameter("prev_a", [BPC, H, W, CIN], F32, isOutput=False)
    w_h = nc.declare_dram_parameter("filter_w", [3, 3, CIN, COUT], F32, isOutput=False)
    b_h = nc.declare_dram_parameter("filter_b", [1, 1, 1, COUT], F32, isOutput=False)
    y_h = nc.declare_dram_parameter("out", [BPC, H, W, COUT], F32, isOutput=True)
    x_ap, w_ap, b_ap, y_ap = x_h.ap(), w_h.ap(), b_h.ap(), y_h.ap()

    with tile.TileContext(nc) as tc, ExitStack() as ctx:
        const_pool = ctx.enter_context(tc.tile_pool(name="const", bufs=1))
        xslab_pool = ctx.enter_context(tc.tile_pool(name="xslab", bufs=1))
        stage_pool = ctx.enter_context(tc.tile_pool(name="stage", bufs=1))
        tmp_pool = ctx.enter_context(tc.tile_pool(name="tmp", bufs=4))
        out_pool = ctx.enter_context(tc.tile_pool(name="outsb", bufs=2))
        psum_mm = ctx.enter_context(
            tc.tile_pool(name="psmm", bufs=4, space=bass.MemorySpace.PSUM)
        )
        psum_wu = ctx.enter_context(
            tc.tile_pool(name="pswu", bufs=1, space=bass.MemorySpace.PSUM)
        )

        # Zeroed region that feeds the PE warm-up matmuls
        warm = const_pool.tile([128, COUT], F16, tag="warm")
        nc.vector.memset(warm[:], 0.0)

        # Stage loads: all four images' [pix, cin] fp16 stages up front (SWDGE
        # casting DMAs), so the load queue never interacts with the pipeline.
        stages = []
        for i in range(BPC):
            stg = stage_pool.tile([CHUNK_PIX, NCHUNK * CIN], F16, tag=f"stg{i}")
            stages.append(stg)
        wslab = const_pool.tile([CIN, 9 * COUT], F16, tag="wslab")

        def emit_load(i, c0, c1):
            src = (
                x_ap[i]
                .rearrange("h w c -> (h w) c")
                .rearrange("(n p) c -> n p c", p=CHUNK_PIX)
                .transpose([1, 0, 2])
            )
            dstv = stages[i][:].rearrange("p (n c) -> p n c", n=NCHUNK)
            nc.gpsimd.dma_start(out=dstv[:, c0:c1, :], in_=src[:, c0:c1, :])

        emit_load(0, 0, 7)
        # Weights: [3,3,128,256] -> SBUF [cin=128, tap*256] fp16 (casting SWDGE)
        nc.gpsimd.dma_start(
            out=wslab[:].rearrange("k (t n) -> k t n", t=9),
            in_=w_ap.rearrange("a b k n -> (a b) k n").transpose([1, 0, 2]),
        )
        emit_load(0, 7, 14)
        emit_load(0, 14, 21)
        emit_load(0, 21, 28)
        for i in range(1, BPC):
            for c0 in range(0, NCHUNK, 7):
                emit_load(i, c0, c0 + 7)

        if with_bias:
            bias_st = const_pool.tile([1, COUT], F32, tag="bias_st")
            nc.sync.dma_start(
                out=bias_st[:], in_=b_ap.rearrange("a b c n -> (a b c) n")
            )
            bias_sb = const_pool.tile([1, COUT], F16, tag="bias")
            nc.vector.tensor_copy(bias_sb[:], bias_st[:])
            ones_sb = const_pool.tile([1, 128], F16, tag="ones")
            nc.gpsimd.memset(ones_sb[:], 1.0)

        # PE warm-up: ~18 junk matmuls release the HAM clock gate (~3.4us of
        # sustained activity) while the first stage DMA + weight cast land.
        wps = psum_wu.tile([128, COUT], F32, tag="pswu")
        NWARM = 18
        for k in range(NWARM):
            nc.tensor.matmul(
                wps[:], warm[:, 0:128], warm[:], start=(k == 0), stop=(k == NWARM - 1)
            )

        # Per-image transposed padded slabs [cin, 58*58 (+slop)]
        xslabs = []
        for i in range(BPC):
            sl = xslab_pool.tile([CIN, SLAB_W], F16, tag=f"xs{i}")
            xslabs.append(sl)
            nc.vector.memset(sl[:, 0:S], 0.0)  # top pad row
            nc.vector.memset(sl[:, (H + 1) * S : PIMG], 0.0)  # bottom pad row
            mid = sl[:, S : (H + 1) * S].rearrange("p (r c) -> p r c", c=S)
            nc.vector.memset(mid[:, :, 0:1], 0.0)  # left pad col
            nc.vector.memset(mid[:, :, S - 1 : S], 0.0)  # right pad col
            nc.vector.memset(sl[:, PIMG:SLAB_W], 0.0)  # slop

        def emit_transpose(i, cidx):
            # stage [112, cin] chunk -> XBAR -> tmp [cin, 112] -> ACT scatter
            # into the padded slab rows (2*cidx, 2*cidx+1)
            tmpt = tmp_pool.tile([CIN, CHUNK_PIX], F16, tag="tmpt")
            nc.sync.dma_start(
                out=tmpt[:],
                in_=stages[i][:, cidx * CIN : (cidx + 1) * CIN],
                transpose=True,
            )
            dst = (
                xslabs[i][:, (RPC * cidx + 1) * S : (RPC * cidx + 1 + RPC) * S]
                .rearrange("p (r c) -> p r c", c=S)[:, :, 1 : 1 + W]
            )
            nc.scalar.activation(
                dst,
                tmpt[:].rearrange("p (r c) -> p r c", c=W),
                mybir.ActivationFunctionType.Copy,
            )

        def emit_window(i, w, oslab):
            q0 = S + 1 + WM * w
            ps = psum_mm.tile([WM, COUT], F32, tag="psmm")
            for t in range(9):
                w0 = q0 + TAP_OFFS[t]
                nc.tensor.matmul(
                    ps[:],
                    xslabs[i][:, w0 : w0 + WM],
                    wslab[:, t * COUT : (t + 1) * COUT],
                    start=(t == 0),
                    stop=(t == 8 and not with_bias),
                )
            if with_bias:
                nc.tensor.matmul(
                    ps[:], ones_sb[:1, :WM], bias_sb[:1, :], start=False, stop=True
                )
            nc.vector.tensor_scalar_max(
                oslab[:, w * COUT : (w + 1) * COUT], ps[:], 0.0
            )

        def emit_stores(i, w, oslab):
            q0 = S + 1 + WM * w
            dst_flat = y_ap[i].rearrange("h w c -> (h w) c")
            for p_off, pix0, ln in _window_runs(q0):
                nc.scalar.dma_start(
                    out=dst_flat[pix0 : pix0 + ln, :],
                    in_=oslab[p_off : p_off + ln, w * COUT : (w + 1) * COUT],
                )

        # Image 0's transposes are emitted up front; image i+1's are
        # interleaved between image i's windows so the ACT queue stays fed.
        for c in range(NCHUNK):
            emit_transpose(0, c)
        for i in range(BPC):
            oslab = out_pool.tile([WM, NW * COUT], F32, tag="osb")
            done = 0
            for w in range(NW):
                emit_window(i, w, oslab)
                emit_stores(i, w, oslab)
                if i + 1 < BPC:
                    want = (w + 1) * NCHUNK // NW
                    while done < want:
                        emit_transpose(i + 1, done)
                        done += 1
            if i + 1 < BPC:
                while done < NCHUNK:
                    emit_transpose(i + 1, done)
                    done += 1

    nc.compile()
    return nc


_CACHE = {}


def _get_nc(with_bias: bool):
    if with_bias not in _CACHE:
        _CACHE[with_bias] = _build(with_bias)
    return _CACHE[with_bias]


def kernel(prev_a, filter_w, filter_b):
    global LAST_RESULTS
    prev_a = np.ascontiguousarray(prev_a, dtype=np.float32)
    filter_w = np.ascontiguousarray(filter_w, dtype=np.float32)
    filter_b = np.ascontiguousarray(filter_b, dtype=np.float32).reshape(1, 1, 1, COUT)
    with_bias = bool(np.any(filter_b))
    nc = _get_nc(with_bias)
    in_maps = [
        {
            "prev_a": prev_a[c * BPC : (c + 1) * BPC],
            "filter_w": filter_w,
            "filter_b": filter_b,
        }
        for c in range(N_CORES)
    ]
    trace = os.environ.get("KERNEL_TRACE") == "1"
    res = run_bass_kernel_spmd(nc, in_maps, list(range(N_CORES)), trace=trace)
    LAST_RESULTS = res
    return np.concatenate([res.results[c]["out"] for c in range(N_CORES)], axis=0)


# revision 7
# speedup vs baseline: 2.4202x; 2.4202x over previous
"""3x3 same-conv (NHWC, 32x56x56x128 -> 32x56x56x256) + bias + ReLU on 8 TRN2 cores.

Strategy: data-parallel over batch (4 images/core). Per core, the conv is
9 shifted matmuls accumulated in PSUM with Cin=128 as the contraction dim:
  out[q, cout] = relu( sum_tap XpT[:, q+off_tap]^T @ W[tap] + b )
where XpT is the zero-padded image held transposed in SBUF ([cin, 58*58]
flat padded pixels, fp16). The anchor axis is tiled in 26 sliding windows
of exactly 128 contiguous padded positions per image (vs 28 row-pair
groups of 116): every matmul uses the full 128 PSUM partitions and the
128-column stationary operand enables fast weight load. Junk anchors
(pad columns) are computed but skipped at store time by splitting each
window into <=3 contiguous valid runs, each stored with its own HWDGE DMA
(contiguous [len,256] fp32 in DRAM).

The input transposes run on the PE (identity-matmul of 2-row stage
chunks; the DMA XBAR alternative measures ~1.2us/chunk and serializes
against other DMAs, starving the PE). A short warm-up accumulation at
t=0 releases the HAM clock gate before the first real matmul. Input
stage loads are casting SWDGE DMAs (fp32->fp16), all issued up front;
output stores go on the ACT HWDGE queue so the SWDGE drain happens
mid-kernel, not in the tail.
"""

import os
from contextlib import ExitStack

import numpy as np

import concourse.bass as bass
import concourse.bacc as bacc
import concourse.mybir as mybir
import concourse.tile as tile
from concourse.bass_utils import run_bass_kernel_spmd
from concourse.masks import make_identity

N_CORES = 8
B, H, W, CIN, COUT = 32, 56, 56, 128, 256
BPC = B // N_CORES            # images per core
S = W + 2                     # padded width (58)
PIMG = S * S                  # padded pixels per image (3364)
NW = 26                       # 128-anchor windows per image
WM = 128                      # anchors per window
SLAB_W = PIMG + 128           # per-image slab width incl. zero slop
RPC = 2                       # image rows per transpose chunk
CHUNK_PIX = RPC * W           # 112
NCHUNK = H // RPC             # 28

TAP_OFFS = [(dh - 1) * S + (dw - 1) for dh in range(3) for dw in range(3)]
F32 = mybir.dt.float32
F16 = mybir.dt.float16

LAST_RESULTS = None


def _window_runs(q0):
    """Contiguous valid-anchor runs inside window [q0, q0+WM).

    Returns (partition_offset, flat_output_pixel, length) triples; anchors
    at padded col 0/57 or pad rows are junk and fall between runs."""
    runs = []
    for r in range(q0 // S, (q0 + WM - 1) // S + 1):
        if r < 1 or r > H:
            continue
        lo = max(q0, r * S + 1)
        hi = min(q0 + WM, r * S + 1 + W)
        if lo < hi:
            runs.append((lo - q0, (r - 1) * W + (lo - r * S - 1), hi - lo))
    return runs


def _build(with_bias: bool):
    nc = bacc.Bacc("TRN2", target_bir_lowering=False, debug=False)
    x_h = nc.declare_dram_parameter("prev_a", [BPC, H, W, CIN], F32, isOutput=False)
    w_h = nc.declare_dram_parameter("filter_w", [3, 3, CIN, COUT], F32, isOutput=False)
    b_h = nc.declare_dram_parameter("filter_b", [1, 1, 1, COUT], F32, isOutput=False)
    y_h = nc.declare_dram_parameter("out", [BPC, H, W, COUT], F32, isOutput=True)
    x_ap, w_ap, b_ap, y_ap = x_h.ap(), w_h.ap(), b_h.ap(), y_h.ap()

    with tile.TileContext(nc) as tc, ExitStack() as ctx:
        const_pool = ctx.enter_context(tc.tile_pool(name="const", bufs=1))
        xslab_pool = ctx.enter_context(tc.tile_pool(name="xslab", bufs=1))
        stage_pool = ctx.enter_context(tc.tile_pool(name="stage", bufs=1))
        out_pool = ctx.enter_context(tc.tile_pool(name="outsb", bufs=2))
        psum_mm = ctx.enter_context(
            tc.tile_pool(name="psmm", bufs=4, space=bass.MemorySpace.PSUM)
        )
        psum_tp = ctx.enter_context(
            tc.tile_pool(name="pstp", bufs=3, space=bass.MemorySpace.PSUM)
        )
        psum_wu = ctx.enter_context(
            tc.tile_pool(name="pswu", bufs=1, space=bass.MemorySpace.PSUM)
        )

        # Zeroed region that feeds the PE warm-up matmuls
        warm = const_pool.tile([128, COUT], F16, tag="warm")
        nc.vector.memset(warm[:], 0.0)

        identity = const_pool.tile([CHUNK_PIX, CHUNK_PIX], F16, tag="ident")
        make_identity(nc, identity[:])

        # Stage loads: all four images' [pix, cin] fp16 stages up front (SWDGE
        # casting DMAs), so the load queue never interacts with the pipeline.
        stages = []
        for i in range(BPC):
            stg = stage_pool.tile([CHUNK_PIX, NCHUNK * CIN], F16, tag=f"stg{i}")
            stages.append(stg)
        wslab = const_pool.tile([CIN, 9 * COUT], F16, tag="wslab")

        def emit_load(i, c0, c1):
            src = (
                x_ap[i]
                .rearrange("h w c -> (h w) c")
                .rearrange("(n p) c -> n p c", p=CHUNK_PIX)
                .transpose([1, 0, 2])
            )
            dstv = stages[i][:].rearrange("p (n c) -> p n c", n=NCHUNK)
            nc.gpsimd.dma_start(out=dstv[:, c0:c1, :], in_=src[:, c0:c1, :])

        emit_load(0, 0, 7)
        # Weights: [3,3,128,256] -> SBUF [cin=128, tap*256] fp16 (casting SWDGE)
        nc.gpsimd.dma_start(
            out=wslab[:].rearrange("k (t n) -> k t n", t=9),
            in_=w_ap.rearrange("a b k n -> (a b) k n").transpose([1, 0, 2]),
        )
        emit_load(0, 7, 14)
        emit_load(0, 14, 21)
        emit_load(0, 21, 28)
        for i in range(1, BPC):
            for c0 in range(0, NCHUNK, 7):
                emit_load(i, c0, c0 + 7)

        if with_bias:
            bias_st = const_pool.tile([1, COUT], F32, tag="bias_st")
            nc.sync.dma_start(
                out=bias_st[:], in_=b_ap.rearrange("a b c n -> (a b c) n")
            )
            bias_sb = const_pool.tile([1, COUT], F16, tag="bias")
            nc.vector.tensor_copy(bias_sb[:], bias_st[:])
            ones_sb = const_pool.tile([1, 128], F16, tag="ones")
            nc.gpsimd.memset(ones_sb[:], 1.0)

        # PE warm-up: ~18 junk matmuls release the HAM clock gate (~3.4us of
        # sustained activity) while the first stage DMA + weight cast land.
        wps = psum_wu.tile([128, COUT], F32, tag="pswu")
        NWARM = 18
        for k in range(NWARM):
            nc.tensor.matmul(
                wps[:], warm[:, 0:128], warm[:], start=(k == 0), stop=(k == NWARM - 1)
            )

        # Per-image transposed padded slabs [cin, 58*58 (+slop)]
        xslabs = []
        for i in range(BPC):
            sl = xslab_pool.tile([CIN, SLAB_W], F16, tag=f"xs{i}")
            xslabs.append(sl)
            nc.vector.memset(sl[:, 0:S], 0.0)  # top pad row
            nc.vector.memset(sl[:, (H + 1) * S : PIMG], 0.0)  # bottom pad row
            mid = sl[:, S : (H + 1) * S].rearrange("p (r c) -> p r c", c=S)
            nc.vector.memset(mid[:, :, 0:1], 0.0)  # left pad col
            nc.vector.memset(mid[:, :, S - 1 : S], 0.0)  # right pad col
            nc.vector.memset(sl[:, PIMG:SLAB_W], 0.0)  # slop

        def emit_transpose(i, cidx):
            # stage [112, cin] chunk -> PE transpose -> PSUM [cin, 112] ->
            # ACT scatter into the padded slab rows (2*cidx, 2*cidx+1)
            pst = psum_tp.tile([CIN, CHUNK_PIX], F16, tag="pst")
            nc.tensor.transpose(
                pst[:], stages[i][:, cidx * CIN : (cidx + 1) * CIN], identity[:]
            )
            dst = (
                xslabs[i][:, (RPC * cidx + 1) * S : (RPC * cidx + 1 + RPC) * S]
                .rearrange("p (r c) -> p r c", c=S)[:, :, 1 : 1 + W]
            )
            nc.scalar.activation(
                dst,
                pst[:].rearrange("p (r c) -> p r c", c=W),
                mybir.ActivationFunctionType.Copy,
            )

        def emit_window(i, w, oslab):
            q0 = S + 1 + WM * w
            ps = psum_mm.tile([WM, COUT], F32, tag="psmm")
            for t in range(9):
                w0 = q0 + TAP_OFFS[t]
                nc.tensor.matmul(
                    ps[:],
                    xslabs[i][:, w0 : w0 + WM],
                    wslab[:, t * COUT : (t + 1) * COUT],
                    start=(t == 0),
                    stop=(t == 8 and not with_bias),
                )
            if with_bias:
                nc.tensor.matmul(
                    ps[:], ones_sb[:1, :WM], bias_sb[:1, :], start=False, stop=True
                )
            nc.vector.tensor_scalar_max(
                oslab[:, w * COUT : (w + 1) * COUT], ps[:], 0.0
            )

        def emit_stores(i, w, oslab):
            q0 = S + 1 + WM * w
            dst_flat = y_ap[i].rearrange("h w c -> (h w) c")
            for p_off, pix0, ln in _window_runs(q0):
                nc.scalar.dma_start(
                    out=dst_flat[pix0 : pix0 + ln, :],
                    in_=oslab[p_off : p_off + ln, w * COUT : (w + 1) * COUT],
                )

        # Image 0's transposes are emitted up front; image i+1's are
        # interleaved between image i's windows so the ACT queue stays fed.
        for c in range(NCHUNK):
            emit_transpose(0, c)
        for i in range(BPC):
            oslab = out_pool.tile([WM, NW * COUT], F32, tag="osb")
            done = 0
            for w in range(NW):
                emit_window(i, w, oslab)
                emit_stores(i, w, oslab)
                if i + 1 < BPC:
                    want = (w + 1) * NCHUNK // NW
                    while done < want:
                        emit_transpose(i + 1, done)
                        done += 1
            if i + 1 < BPC:
                while done < NCHUNK:
                    emit_transpose(i + 1, done)
                    done += 1

    nc.compile()
    return nc


_CACHE = {}


def _get_nc(with_bias: bool):
    if with_bias not in _CACHE:
        _CACHE[with_bias] = _build(with_bias)
    return _CACHE[with_bias]


def kernel(prev_a, filter_w, filter_b):
    global LAST_RESULTS
    prev_a = np.ascontiguousarray(prev_a, dtype=np.float32)
    filter_w = np.ascontiguousarray(filter_w, dtype=np.float32)
    filter_b = np.ascontiguousarray(filter_b, dtype=np.float32).reshape(1, 1, 1, COUT)
    with_bias = bool(np.any(filter_b))
    nc = _get_nc(with_bias)
    in_maps = [
        {
            "prev_a": prev_a[c * BPC : (c + 1) * BPC],
            "filter_w": filter_w,
            "filter_b": filter_b,
        }
        for c in range(N_CORES)
    ]
    trace = os.environ.get("KERNEL_TRACE") == "1"
    res = run_bass_kernel_spmd(nc, in_maps, list(range(N_CORES)), trace=trace)
    LAST_RESULTS = res
    return np.concatenate([res.results[c]["out"] for c in range(N_CORES)], axis=0)


# revision 12
# speedup vs baseline: 5.2892x; 2.1854x over previous
"""3x3 same-conv (NHWC, 32x56x56x128 -> 32x56x56x256) + bias + ReLU on 8 TRN2 cores.

Strategy: data-parallel over batch (4 images/core). Per core, the conv is
9 shifted matmuls accumulated in PSUM with Cin=128 as the contraction dim
over a PACKED 56-wide slab: XpT[cin, p] with p(r,c) = (r+1)*56+c+1, only
vertical pad rows (no left/right pad columns). The anchor axis is then a
dense [57, 3193) range tiled in 25 windows of exactly 128 anchors: every
matmul runs with the full 128 PSUM partitions / 128-column stationary
operand (fast weight load), there are zero junk anchors, and the store is
a handful of large regular DMAs per image ([128, nw, 256] window-major ->
pixel-major DRAM).

The packed layout makes the horizontal taps WRAP at row edges, so output
columns 0 and 55 are recomputed by a small edge pass: four column strips
(cols 0,55,1,54) are copied into a column-major mini-slab [cin, 4*58],
12 matmuls of M=56 rebuild the two edge columns exactly, and their stores
overwrite the wrapped values (DRAM write-after-write order is enforced by
the tile framework's shadow-memory deps plus SWDGE queue FIFO).

Input transposes run on the PE (identity-matmul of 2-row stage chunks;
the DMA-XBAR alternative measures ~1.2us/chunk and serializes against
other DMAs). A warm-up burst of identity transposes at t=0 releases the
HAM clock gate before the first real matmul. Input stage loads are
casting SWDGE DMAs (fp32->fp16) issued up front.
"""

import os
from contextlib import ExitStack

import numpy as np

import concourse.bass as bass
import concourse.bacc as bacc
import concourse.mybir as mybir
import concourse.tile as tile
from concourse.bass_utils import run_bass_kernel_spmd
from concourse.masks import make_identity

N_CORES = 8
B, H, W, CIN, COUT = 32, 56, 56, 128, 256
BPC = B // N_CORES            # images per core
PIX = H * W                   # 3136
SLAB_W = 3328                 # 1 + pad row + 56 rows + pad row + slop
ABASE = W + 1                 # first anchor (pixel (0,0) at 57)
NW = 25                       # 128-anchor windows per image
WM = 128                      # anchors per window
RPC = 2                       # image rows per transpose chunk
CHUNK_PIX = RPC * W           # 112
NCHUNK = H // RPC             # 28
ES = H + 2                    # edge strip length (58)

TAP_OFFS = [(dh - 1) * W + (dw - 1) for dh in range(3) for dw in range(3)]
F32 = mybir.dt.float32
F16 = mybir.dt.float16

LAST_RESULTS = None


def _build(with_bias: bool):
    nc = bacc.Bacc("TRN2", target_bir_lowering=False, debug=False)
    x_h = nc.declare_dram_parameter("prev_a", [BPC, H, W, CIN], F32, isOutput=False)
    w_h = nc.declare_dram_parameter("filter_w", [3, 3, CIN, COUT], F32, isOutput=False)
    b_h = nc.declare_dram_parameter("filter_b", [1, 1, 1, COUT], F32, isOutput=False)
    y_h = nc.declare_dram_parameter("out", [BPC, H, W, COUT], F32, isOutput=True)
    x_ap, w_ap, b_ap, y_ap = x_h.ap(), w_h.ap(), b_h.ap(), y_h.ap()

    with tile.TileContext(nc) as tc, ExitStack() as ctx:
        const_pool = ctx.enter_context(tc.tile_pool(name="const", bufs=1))
        xslab_pool = ctx.enter_context(tc.tile_pool(name="xslab", bufs=1))
        stage_pool = ctx.enter_context(tc.tile_pool(name="stage", bufs=1))
        edge_pool = ctx.enter_context(tc.tile_pool(name="edge", bufs=2))
        out_pool = ctx.enter_context(tc.tile_pool(name="outsb", bufs=2))
        psum_mm = ctx.enter_context(
            tc.tile_pool(name="psmm", bufs=3, space=bass.MemorySpace.PSUM)
        )
        psum_tp = ctx.enter_context(
            tc.tile_pool(name="pstp", bufs=3, space=bass.MemorySpace.PSUM)
        )
        psum_ed = ctx.enter_context(
            tc.tile_pool(name="psed", bufs=2, space=bass.MemorySpace.PSUM)
        )

        identity = const_pool.tile([CHUNK_PIX, CHUNK_PIX], F16, tag="ident")
        make_identity(nc, identity[:])

        # Stage tiles + loads: all four images' [pix, cin] fp16 stages up
        # front (casting SWDGE DMAs); the weight cast rides between the first
        # image's chunks so the first window can start ASAP.
        stages = [
            stage_pool.tile(
                [CHUNK_PIX, NCHUNK * CIN], F16, tag=f"stg{i}", name=f"stg{i}"
            )
            for i in range(BPC)
        ]
        wslab = const_pool.tile([CIN, 9 * COUT], F16, tag="wslab")

        def emit_load(i, c0, c1):
            src = (
                x_ap[i]
                .rearrange("h w c -> (h w) c")
                .rearrange("(n p) c -> n p c", p=CHUNK_PIX)
                .transpose([1, 0, 2])
            )
            dstv = stages[i][:].rearrange("p (n c) -> p n c", n=NCHUNK)
            nc.gpsimd.dma_start(out=dstv[:, c0:c1, :], in_=src[:, c0:c1, :])

        emit_load(0, 0, 7)
        nc.gpsimd.dma_start(
            out=wslab[:].rearrange("k (t n) -> k t n", t=9),
            in_=w_ap.rearrange("a b k n -> (a b) k n").transpose([1, 0, 2]),
        )
        emit_load(0, 7, 14)
        emit_load(0, 14, 21)
        emit_load(0, 21, 28)
        for i in range(1, BPC):
            for c0 in range(0, NCHUNK, 7):
                emit_load(i, c0, c0 + 7)

        if with_bias:
            bias_st = const_pool.tile([1, COUT], F32, tag="bias_st")
            nc.sync.dma_start(
                out=bias_st[:], in_=b_ap.rearrange("a b c n -> (a b c) n")
            )
            bias_sb = const_pool.tile([1, COUT], F16, tag="bias")
            nc.vector.tensor_copy(bias_sb[:], bias_st[:])
            ones_sb = const_pool.tile([1, 128], F16, tag="ones")
            nc.gpsimd.memset(ones_sb[:], 1.0)

        # PE warm-up: ~30 junk identity transposes keep the PE busy through
        # the HAM activity window (~3.4us) while the first DMAs land, so the
        # first real matmul runs at the warm 2.4 GHz clock.
        for _ in range(30):
            pwu = psum_tp.tile([CIN, CHUNK_PIX], F16, tag="pst")
            nc.tensor.transpose(
                pwu[0:CHUNK_PIX, 0:CHUNK_PIX], identity[:], identity[:]
            )

        # Per-image packed transposed slabs [cin, 1 + 58*56 rows + slop]
        xslabs = []
        for i in range(BPC):
            sl = xslab_pool.tile([CIN, SLAB_W], F16, tag=f"xs{i}")
            xslabs.append(sl)
            nc.vector.memset(sl[:, 0 : ABASE], 0.0)           # lead + top pad row
            nc.vector.memset(sl[:, (H + 1) * W + 1 : SLAB_W], 0.0)  # bottom pad + slop

        def emit_transpose(i, cidx):
            # stage [112, cin] chunk -> PE transpose -> PSUM [cin, 112] ->
            # ACT copy into the packed slab (contiguous 112 span)
            pst = psum_tp.tile([CIN, CHUNK_PIX], F16, tag="pst")
            nc.tensor.transpose(
                pst[:], stages[i][:, cidx * CIN : (cidx + 1) * CIN], identity[:]
            )
            d0 = (RPC * cidx + 1) * W + 1
            nc.scalar.activation(
                xslabs[i][:, d0 : d0 + CHUNK_PIX],
                pst[:],
                mybir.ActivationFunctionType.Copy,
            )

        def emit_ebuild(i):
            # column-major edge mini-slab: strips [col0 | col55 | col1 | col54]
            ed = edge_pool.tile([CIN, 4 * ES], F16, tag="E")
            for s, col in enumerate([0, 55, 1, 54]):
                nc.vector.memset(ed[:, s * ES : s * ES + 1], 0.0)
                nc.vector.memset(ed[:, s * ES + H + 1 : (s + 1) * ES], 0.0)
                src = (
                    xslabs[i][:, W + 1 + col : W + 1 + col + H * W]
                    .rearrange("p (r c) -> p r c", c=W)[:, :, 0:1]
                    .rearrange("p r c -> p (r c)")
                )
                nc.scalar.activation(
                    ed[:, s * ES + 1 : s * ES + 1 + H],
                    src,
                    mybir.ActivationFunctionType.Copy,
                )
            return ed

        def emit_window(i, w, oslab):
            q0 = ABASE + WM * w
            ps = psum_mm.tile([WM, COUT], F32, tag="psmm")
            for t in range(9):
                w0 = q0 + TAP_OFFS[t]
                nc.tensor.matmul(
                    ps[:],
                    xslabs[i][:, w0 : w0 + WM],
                    wslab[:, t * COUT : (t + 1) * COUT],
                    start=(t == 0),
                    stop=(t == 8 and not with_bias),
                )
            if with_bias:
                nc.tensor.matmul(
                    ps[:], ones_sb[:1, :WM], bias_sb[:1, :], start=False, stop=True
                )
            nc.vector.tensor_scalar_max(
                oslab[:, w * COUT : (w + 1) * COUT], ps[:], 0.0
            )

        # main-store chunk boundaries (after these windows' relu)
        STORE_AT = {6: (0, 7), 13: (7, 14), 19: (14, 20), 23: (20, 24)}

        def emit_store_chunk(i, oslab, w0, w1):
            dst = (
                y_ap[i]
                .rearrange("h w c -> (h w) c")[w0 * WM : w1 * WM, :]
                .rearrange("(w p) c -> p w c", p=WM)
            )
            src = oslab[:, w0 * COUT : w1 * COUT].rearrange(
                "p (w k) -> p w k", k=COUT
            )
            nc.gpsimd.dma_start(out=dst, in_=src)

        def emit_store_last(i, oslab):
            n = PIX - 24 * WM  # 64
            dst = y_ap[i].rearrange("h w c -> (h w) c")[24 * WM :, :]
            nc.gpsimd.dma_start(out=dst, in_=oslab[0:n, 24 * COUT : 25 * COUT])

        # edge pass: 2x6 matmuls of M=56 rebuild output cols 0 / 55 exactly;
        # their stores overwrite the wrapped main-store values.
        EL_TAPS = [(0, 1), (2, 2)]  # (strip, dw): col0 reads col0(dw=1), col1(dw=2)
        ER_TAPS = [(1, 1), (3, 0)]  # col55 reads col55(dw=1), col54(dw=0)

        def emit_edge(i, ed):
            for side, taps, col in ((0, EL_TAPS, 0), (1, ER_TAPS, 55)):
                pe = psum_ed.tile([H, COUT], F32, tag="psed")
                k = 0
                nmm = 6 + (1 if with_bias else 0)
                for s, dw in taps:
                    for dh in range(3):
                        t = dh * 3 + dw
                        nc.tensor.matmul(
                            pe[:],
                            ed[:, s * ES + dh : s * ES + dh + H],
                            wslab[:, t * COUT : (t + 1) * COUT],
                            start=(k == 0),
                            stop=(k == nmm - 1),
                        )
                        k += 1
                if with_bias:
                    nc.tensor.matmul(
                        pe[:], ones_sb[:1, :H], bias_sb[:1, :], start=False, stop=True
                    )
                esb = edge_pool.tile([H, COUT], F32, tag=f"esb{side}")
                nc.vector.tensor_scalar_max(esb[:], pe[:], 0.0)
                nc.gpsimd.dma_start(out=y_ap[i][:, col, :], in_=esb[:])

        # Image 0's transposes up front; image i+1's are interleaved between
        # image i's windows so the PE never waits on a bulk transpose phase.
        for c in range(NCHUNK):
            emit_transpose(0, c)
        ed = emit_ebuild(0)
        for i in range(BPC):
            oslab = out_pool.tile([WM, NW * COUT], F32, tag="osb")
            done = 0
            for w in range(NW):
                emit_window(i, w, oslab)
                if w in STORE_AT:
                    emit_store_chunk(i, oslab, *STORE_AT[w])
                if i + 1 < BPC:
                    want = (w + 1) * NCHUNK // NW
                    while done < want:
                        emit_transpose(i + 1, done)
                        done += 1
            emit_store_last(i, oslab)
            if i + 1 < BPC:
                while done < NCHUNK:
                    emit_transpose(i + 1, done)
                    done += 1
            next_ed = emit_ebuild(i + 1) if i + 1 < BPC else None
            emit_edge(i, ed)
            ed = next_ed

    nc.compile()
    return nc


_CACHE = {}


def _get_nc(with_bias: bool):
    if with_bias not in _CACHE:
        _CACHE[with_bias] = _build(with_bias)
    return _CACHE[with_bias]


def kernel(prev_a, filter_w, filter_b):
    global LAST_RESULTS
    prev_a = np.ascontiguousarray(prev_a, dtype=np.float32)
    filter_w = np.ascontiguousarray(filter_w, dtype=np.float32)
    filter_b = np.ascontiguousarray(filter_b, dtype=np.float32).reshape(1, 1, 1, COUT)
    with_bias = bool(np.any(filter_b))
    nc = _get_nc(with_bias)
    in_maps = [
        {
            "prev_a": prev_a[c * BPC : (c + 1) * BPC],
            "filter_w": filter_w,
            "filter_b": filter_b,
        }
        for c in range(N_CORES)
    ]
    trace = os.environ.get("KERNEL_TRACE") == "1"
    res = run_bass_kernel_spmd(nc, in_maps, list(range(N_CORES)), trace=trace)
    LAST_RESULTS = res
    return np.concatenate([res.results[c]["out"] for c in range(N_CORES)], axis=0)


# revision 13
# speedup vs baseline: 5.3445x; 1.0104x over previous
"""3x3 same-conv (NHWC, 32x56x56x128 -> 32x56x56x256) + bias + ReLU on 8 TRN2 cores.

Strategy: data-parallel over batch (4 images/core). Per core, the conv is
9 shifted matmuls accumulated in PSUM with Cin=128 as the contraction dim
over a PACKED 56-wide slab: XpT[cin, p] with p(r,c) = (r+1)*56+c+1, only
vertical pad rows (no left/right pad columns). The anchor axis is then a
dense [57, 3193) range tiled in 25 windows of exactly 128 anchors: every
matmul runs with the full 128 PSUM partitions / 128-column stationary
operand (fast weight load), there are zero junk anchors, and the store is
a handful of large regular DMAs per image ([128, nw, 256] window-major ->
pixel-major DRAM).

The packed layout makes the horizontal taps WRAP at row edges, so output
columns 0 and 55 are recomputed by a small edge pass: four column strips
(cols 0,55,1,54) are copied into a column-major mini-slab [cin, 4*58],
12 matmuls of M=56 rebuild the two edge columns exactly, and their stores
overwrite the wrapped values (DRAM write-after-write order is enforced by
the tile framework's shadow-memory deps plus SWDGE queue FIFO).

Input transposes run on the PE (identity-matmul of 2-row stage chunks;
the DMA-XBAR alternative measures ~1.2us/chunk and serializes against
other DMAs). A warm-up burst of identity transposes at t=0 releases the
HAM clock gate before the first real matmul. Input stage loads are
casting SWDGE DMAs (fp32->fp16) issued up front.
"""

import os
from contextlib import ExitStack

import numpy as np

import concourse.bass as bass
import concourse.bacc as bacc
import concourse.mybir as mybir
import concourse.tile as tile
from concourse.bass_utils import run_bass_kernel_spmd
from concourse.masks import make_identity

N_CORES = 8
B, H, W, CIN, COUT = 32, 56, 56, 128, 256
BPC = B // N_CORES            # images per core
PIX = H * W                   # 3136
SLAB_W = 3328                 # 1 + pad row + 56 rows + pad row + slop
ABASE = W + 1                 # first anchor (pixel (0,0) at 57)
NW = 25                       # 128-anchor windows per image
WM = 128                      # anchors per window
RPC = 2                       # image rows per transpose chunk
CHUNK_PIX = RPC * W           # 112
NCHUNK = H // RPC             # 28
ES = H + 2                    # edge strip length (58)

TAP_OFFS = [(dh - 1) * W + (dw - 1) for dh in range(3) for dw in range(3)]
F32 = mybir.dt.float32
F16 = mybir.dt.float16

LAST_RESULTS = None


def _build(with_bias: bool):
    nc = bacc.Bacc("TRN2", target_bir_lowering=False, debug=False)
    x_h = nc.declare_dram_parameter("prev_a", [BPC, H, W, CIN], F32, isOutput=False)
    w_h = nc.declare_dram_parameter("filter_w", [3, 3, CIN, COUT], F32, isOutput=False)
    b_h = nc.declare_dram_parameter("filter_b", [1, 1, 1, COUT], F32, isOutput=False)
    y_h = nc.declare_dram_parameter("out", [BPC, H, W, COUT], F32, isOutput=True)
    x_ap, w_ap, b_ap, y_ap = x_h.ap(), w_h.ap(), b_h.ap(), y_h.ap()

    with tile.TileContext(nc) as tc, ExitStack() as ctx:
        const_pool = ctx.enter_context(tc.tile_pool(name="const", bufs=1))
        xslab_pool = ctx.enter_context(tc.tile_pool(name="xslab", bufs=1))
        stage_pool = ctx.enter_context(tc.tile_pool(name="stage", bufs=1))
        edge_pool = ctx.enter_context(tc.tile_pool(name="edge", bufs=2))
        out_pool = ctx.enter_context(tc.tile_pool(name="outsb", bufs=2))
        psum_mm = ctx.enter_context(
            tc.tile_pool(name="psmm", bufs=3, space=bass.MemorySpace.PSUM)
        )
        psum_tp = ctx.enter_context(
            tc.tile_pool(name="pstp", bufs=3, space=bass.MemorySpace.PSUM)
        )
        psum_ed = ctx.enter_context(
            tc.tile_pool(name="psed", bufs=2, space=bass.MemorySpace.PSUM)
        )

        identity = const_pool.tile([CHUNK_PIX, CHUNK_PIX], F16, tag="ident")
        make_identity(nc, identity[:])

        # Stage tiles + loads: all four images' [pix, cin] fp16 stages up
        # front (casting SWDGE DMAs); the weight cast rides between the first
        # image's chunks so the first window can start ASAP.
        stages = [
            stage_pool.tile(
                [CHUNK_PIX, NCHUNK * CIN], F16, tag=f"stg{i}", name=f"stg{i}"
            )
            for i in range(BPC)
        ]
        wslab = const_pool.tile([CIN, 9 * COUT], F16, tag="wslab")

        def emit_load(i, c0, c1):
            src = (
                x_ap[i]
                .rearrange("h w c -> (h w) c")
                .rearrange("(n p) c -> n p c", p=CHUNK_PIX)
                .transpose([1, 0, 2])
            )
            dstv = stages[i][:].rearrange("p (n c) -> p n c", n=NCHUNK)
            nc.gpsimd.dma_start(out=dstv[:, c0:c1, :], in_=src[:, c0:c1, :])

        # Weights ride the (otherwise idle) HWDGE queue as fp32 + DVE cast;
        # a casting SWDGE load here would serialize behind the stage loads
        # and gate the first conv matmul (~8us later, measured).
        wstage = const_pool.tile([CIN, 9 * COUT], F32, tag="wstage")
        nc.sync.dma_start(
            out=wstage[:].rearrange("k (t n) -> k t n", t=9),
            in_=w_ap.rearrange("a b k n -> (a b) k n").transpose([1, 0, 2]),
        )
        nc.vector.tensor_copy(wslab[:], wstage[:])

        emit_load(0, 0, 2)
        emit_load(0, 2, 7)
        emit_load(0, 7, 14)
        emit_load(0, 14, 21)
        emit_load(0, 21, 28)
        for i in range(1, BPC):
            for c0 in range(0, NCHUNK, 7):
                emit_load(i, c0, c0 + 7)

        if with_bias:
            bias_st = const_pool.tile([1, COUT], F32, tag="bias_st")
            nc.sync.dma_start(
                out=bias_st[:], in_=b_ap.rearrange("a b c n -> (a b c) n")
            )
            bias_sb = const_pool.tile([1, COUT], F16, tag="bias")
            nc.vector.tensor_copy(bias_sb[:], bias_st[:])
            ones_sb = const_pool.tile([1, 128], F16, tag="ones")
            nc.gpsimd.memset(ones_sb[:], 1.0)

        # PE warm-up: ~30 junk identity transposes keep the PE busy through
        # the HAM activity window (~3.4us) while the first DMAs land, so the
        # first real matmul runs at the warm 2.4 GHz clock.
        for _ in range(30):
            pwu = psum_tp.tile([CIN, CHUNK_PIX], F16, tag="pst")
            nc.tensor.transpose(
                pwu[0:CHUNK_PIX, 0:CHUNK_PIX], identity[:], identity[:]
            )

        # Per-image packed transposed slabs [cin, 1 + 58*56 rows + slop]
        xslabs = []
        for i in range(BPC):
            sl = xslab_pool.tile([CIN, SLAB_W], F16, tag=f"xs{i}")
            xslabs.append(sl)
            nc.vector.memset(sl[:, 0 : ABASE], 0.0)           # lead + top pad row
            nc.vector.memset(sl[:, (H + 1) * W + 1 : SLAB_W], 0.0)  # bottom pad + slop

        def emit_transpose(i, cidx):
            # stage [112, cin] chunk -> PE transpose -> PSUM [cin, 112] ->
            # ACT copy into the packed slab (contiguous 112 span)
            pst = psum_tp.tile([CIN, CHUNK_PIX], F16, tag="pst")
            nc.tensor.transpose(
                pst[:], stages[i][:, cidx * CIN : (cidx + 1) * CIN], identity[:]
            )
            d0 = (RPC * cidx + 1) * W + 1
            nc.scalar.activation(
                xslabs[i][:, d0 : d0 + CHUNK_PIX],
                pst[:],
                mybir.ActivationFunctionType.Copy,
            )

        def emit_ebuild(i):
            # column-major edge mini-slab: strips [col0 | col55 | col1 | col54]
            ed = edge_pool.tile([CIN, 4 * ES], F16, tag="E")
            for s, col in enumerate([0, 55, 1, 54]):
                nc.vector.memset(ed[:, s * ES : s * ES + 1], 0.0)
                nc.vector.memset(ed[:, s * ES + H + 1 : (s + 1) * ES], 0.0)
                src = (
                    xslabs[i][:, W + 1 + col : W + 1 + col + H * W]
                    .rearrange("p (r c) -> p r c", c=W)[:, :, 0:1]
                    .rearrange("p r c -> p (r c)")
                )
                nc.scalar.activation(
                    ed[:, s * ES + 1 : s * ES + 1 + H],
                    src,
                    mybir.ActivationFunctionType.Copy,
                )
            return ed

        def emit_window(i, w, oslab):
            q0 = ABASE + WM * w
            ps = psum_mm.tile([WM, COUT], F32, tag="psmm")
            for t in range(9):
                w0 = q0 + TAP_OFFS[t]
                nc.tensor.matmul(
                    ps[:],
                    xslabs[i][:, w0 : w0 + WM],
                    wslab[:, t * COUT : (t + 1) * COUT],
                    start=(t == 0),
                    stop=(t == 8 and not with_bias),
                )
            if with_bias:
                nc.tensor.matmul(
                    ps[:], ones_sb[:1, :WM], bias_sb[:1, :], start=False, stop=True
                )
            nc.vector.tensor_scalar_max(
                oslab[:, w * COUT : (w + 1) * COUT], ps[:], 0.0
            )

        # main-store chunk boundaries (after these windows' relu)
        STORE_AT = {6: (0, 7), 13: (7, 14), 19: (14, 20), 23: (20, 24)}

        def emit_store_chunk(i, oslab, w0, w1):
            dst = (
                y_ap[i]
                .rearrange("h w c -> (h w) c")[w0 * WM : w1 * WM, :]
                .rearrange("(w p) c -> p w c", p=WM)
            )
            src = oslab[:, w0 * COUT : w1 * COUT].rearrange(
                "p (w k) -> p w k", k=COUT
            )
            nc.gpsimd.dma_start(out=dst, in_=src)

        def emit_store_last(i, oslab):
            n = PIX - 24 * WM  # 64
            dst = y_ap[i].rearrange("h w c -> (h w) c")[24 * WM :, :]
            nc.gpsimd.dma_start(out=dst, in_=oslab[0:n, 24 * COUT : 25 * COUT])

        # edge pass: 2x6 matmuls of M=56 rebuild output cols 0 / 55 exactly;
        # their stores overwrite the wrapped main-store values.
        EL_TAPS = [(0, 1), (2, 2)]  # (strip, dw): col0 reads col0(dw=1), col1(dw=2)
        ER_TAPS = [(1, 1), (3, 0)]  # col55 reads col55(dw=1), col54(dw=0)

        def emit_edge(i, ed):
            for side, taps, col in ((0, EL_TAPS, 0), (1, ER_TAPS, 55)):
                pe = psum_ed.tile([H, COUT], F32, tag="psed")
                k = 0
                nmm = 6 + (1 if with_bias else 0)
                for s, dw in taps:
                    for dh in range(3):
                        t = dh * 3 + dw
                        nc.tensor.matmul(
                            pe[:],
                            ed[:, s * ES + dh : s * ES + dh + H],
                            wslab[:, t * COUT : (t + 1) * COUT],
                            start=(k == 0),
                            stop=(k == nmm - 1),
                        )
                        k += 1
                if with_bias:
                    nc.tensor.matmul(
                        pe[:], ones_sb[:1, :H], bias_sb[:1, :], start=False, stop=True
                    )
                esb = edge_pool.tile([H, COUT], F32, tag=f"esb{side}")
                nc.vector.tensor_scalar_max(esb[:], pe[:], 0.0)
                nc.gpsimd.dma_start(out=y_ap[i][:, col, :], in_=esb[:])

        # Image 0's transposes up front; image i+1's are interleaved between
        # image i's windows so the PE never waits on a bulk transpose phase.
        for c in range(NCHUNK):
            emit_transpose(0, c)
        ed = emit_ebuild(0)
        for i in range(BPC):
            oslab = out_pool.tile([WM, NW * COUT], F32, tag="osb")
            done = 0
            for w in range(NW):
                emit_window(i, w, oslab)
                if w in STORE_AT:
                    emit_store_chunk(i, oslab, *STORE_AT[w])
                if i + 1 < BPC:
                    want = (w + 1) * NCHUNK // NW
                    while done < want:
                        emit_transpose(i + 1, done)
                        done += 1
            emit_store_last(i, oslab)
            if i + 1 < BPC:
                while done < NCHUNK:
                    emit_transpose(i + 1, done)
                    done += 1
            next_ed = emit_ebuild(i + 1) if i + 1 < BPC else None
            emit_edge(i, ed)
            ed = next_ed

    nc.compile()
    return nc


_CACHE = {}


def _get_nc(with_bias: bool):
    if with_bias not in _CACHE:
        _CACHE[with_bias] = _build(with_bias)
    return _CACHE[with_bias]


def kernel(prev_a, filter_w, filter_b):
    global LAST_RESULTS
    prev_a = np.ascontiguousarray(prev_a, dtype=np.float32)
    filter_w = np.ascontiguousarray(filter_w, dtype=np.float32)
    filter_b = np.ascontiguousarray(filter_b, dtype=np.float32).reshape(1, 1, 1, COUT)
    with_bias = bool(np.any(filter_b))
    nc = _get_nc(with_bias)
    in_maps = [
        {
            "prev_a": prev_a[c * BPC : (c + 1) * BPC],
            "filter_w": filter_w,
            "filter_b": filter_b,
        }
        for c in range(N_CORES)
    ]
    trace = os.environ.get("KERNEL_TRACE") == "1"
    res = run_bass_kernel_spmd(nc, in_maps, list(range(N_CORES)), trace=trace)
    LAST_RESULTS = res
    return np.concatenate([res.results[c]["out"] for c in range(N_CORES)], axis=0)


# revision 28
# speedup vs baseline: 5.3719x; 1.0051x over previous
"""3x3 same-conv (NHWC, 32x56x56x128 -> 32x56x56x256) + bias + ReLU on 8 TRN2 cores.

Strategy: data-parallel over batch (4 images/core). Per core, the conv is
9 shifted matmuls accumulated in PSUM with Cin=128 as the contraction dim
over a PACKED 56-wide slab: XpT[cin, p] with p(r,c) = (r+1)*56+c+1, only
vertical pad rows (no left/right pad columns). The anchor axis is then a
dense [57, 3193) range tiled in 25 windows of exactly 128 anchors: every
matmul runs with the full 128 PSUM partitions / 128-column stationary
operand (fast weight load), there are zero junk anchors, and the store is
a handful of large regular DMAs per image ([128, nw, 256] window-major ->
pixel-major DRAM).

The packed layout makes the horizontal taps WRAP at row edges, so output
columns 0 and 55 are recomputed by a small edge pass: four column strips
(cols 0,55,1,54) are copied into a column-major mini-slab [cin, 4*58],
12 matmuls of M=56 rebuild the two edge columns exactly, and their stores
overwrite the wrapped values (DRAM write-after-write order is enforced by
the tile framework's shadow-memory deps plus SWDGE queue FIFO).

Input transposes run on the PE (identity-matmul of 2-row stage chunks;
the DMA-XBAR alternative measures ~1.2us/chunk and serializes against
other DMAs). A warm-up burst of identity transposes at t=0 releases the
HAM clock gate before the first real matmul. Input stage loads are
casting SWDGE DMAs (fp32->fp16) issued up front.
"""

import os
from contextlib import ExitStack

import numpy as np

import concourse.bass as bass
import concourse.bacc as bacc
import concourse.mybir as mybir
import concourse.tile as tile
from concourse.bass_utils import run_bass_kernel_spmd
from concourse.masks import make_identity

N_CORES = 8
B, H, W, CIN, COUT = 32, 56, 56, 128, 256
BPC = B // N_CORES            # images per core
PIX = H * W                   # 3136
SLAB_W = 3328                 # 1 + pad row + 56 rows + pad row + slop
ABASE = W + 1                 # first anchor (pixel (0,0) at 57)
NW = 25                       # 128-anchor windows per image
WM = 128                      # anchors per window
RPC = 2                       # image rows per transpose chunk
CHUNK_PIX = RPC * W           # 112
NCHUNK = H // RPC             # 28
ES = H + 2                    # edge strip length (58)

TAP_OFFS = [(dh - 1) * W + (dw - 1) for dh in range(3) for dw in range(3)]
F32 = mybir.dt.float32
F16 = mybir.dt.float16

LAST_RESULTS = None


def _build(with_bias: bool):
    nc = bacc.Bacc("TRN2", target_bir_lowering=False, debug=False)
    x_h = nc.declare_dram_parameter("prev_a", [BPC, H, W, CIN], F32, isOutput=False)
    w_h = nc.declare_dram_parameter("filter_w", [3, 3, CIN, COUT], F32, isOutput=False)
    b_h = nc.declare_dram_parameter("filter_b", [1, 1, 1, COUT], F32, isOutput=False)
    y_h = nc.declare_dram_parameter("out", [BPC, H, W, COUT], F32, isOutput=True)
    x_ap, w_ap, b_ap, y_ap = x_h.ap(), w_h.ap(), b_h.ap(), y_h.ap()

    with tile.TileContext(nc) as tc, ExitStack() as ctx:
        const_pool = ctx.enter_context(tc.tile_pool(name="const", bufs=1))
        xslab_pool = ctx.enter_context(tc.tile_pool(name="xslab", bufs=1))
        stage_pool = ctx.enter_context(tc.tile_pool(name="stage", bufs=1))
        edge_pool = ctx.enter_context(tc.tile_pool(name="edge", bufs=2))
        out_pool = ctx.enter_context(tc.tile_pool(name="outsb", bufs=2))
        psum_mm = ctx.enter_context(
            tc.tile_pool(name="psmm", bufs=3, space=bass.MemorySpace.PSUM)
        )
        psum_tp = ctx.enter_context(
            tc.tile_pool(name="pstp", bufs=3, space=bass.MemorySpace.PSUM)
        )
        psum_ed = ctx.enter_context(
            tc.tile_pool(name="psed", bufs=2, space=bass.MemorySpace.PSUM)
        )

        identity = const_pool.tile([CHUNK_PIX, CHUNK_PIX], F16, tag="ident")
        make_identity(nc, identity[:])

        # Stage tiles + loads: all four images' [pix, cin] fp16 stages up
        # front (casting SWDGE DMAs); the weight cast rides between the first
        # image's chunks so the first window can start ASAP.
        stages = [
            stage_pool.tile(
                [CHUNK_PIX, NCHUNK * CIN], F16, tag=f"stg{i}", name=f"stg{i}"
            )
            for i in range(BPC)
        ]
        wslab = const_pool.tile([CIN, 9 * COUT], F16, tag="wslab")

        def emit_load(i, c0, c1):
            src = (
                x_ap[i]
                .rearrange("h w c -> (h w) c")
                .rearrange("(n p) c -> n p c", p=CHUNK_PIX)
                .transpose([1, 0, 2])
            )
            dstv = stages[i][:].rearrange("p (n c) -> p n c", n=NCHUNK)
            nc.gpsimd.dma_start(out=dstv[:, c0:c1, :], in_=src[:, c0:c1, :])

        # Weights ride the (otherwise idle) HWDGE queue as fp32 + DVE cast;
        # a casting SWDGE load here would serialize behind the stage loads
        # and gate the first conv matmul (~8us later, measured).
        wstage = const_pool.tile([CIN, 9 * COUT], F32, tag="wstage")
        wsrc = w_ap.rearrange("a b k n -> (a b) k n").transpose([1, 0, 2])
        wdst = wstage[:].rearrange("k (t n) -> k t n", t=9)

        # Head start: image 0's first 6 chunks ride the HWDGE queue as fp32
        # (interleaved with the weights) and are cast to the stage by DVE --
        # the SWDGE casting transfers are too slow to feed the PE by ~7us.
        HS = 6
        stage32 = const_pool.tile([CHUNK_PIX, HS * CIN], F32, tag="stage32")
        hs_src = (
            x_ap[0]
            .rearrange("h w c -> (h w) c")
            .rearrange("(n p) c -> n p c", p=CHUNK_PIX)
            .transpose([1, 0, 2])
        )
        hs_dst = stage32[:].rearrange("p (n c) -> p n c", n=HS)
        st_dst = stages[0][:].rearrange("p (n c) -> p n c", n=NCHUNK)
        nc.sync.dma_start(out=hs_dst[:, 0:2, :], in_=hs_src[:, 0:2, :])
        nc.vector.tensor_copy(st_dst[:, 0:2, :], hs_dst[:, 0:2, :])
        nc.sync.dma_start(out=wdst[:, 0:5, :], in_=wsrc[:, 0:5, :])
        nc.vector.tensor_copy(wslab[:, : 5 * COUT], wstage[:, : 5 * COUT])
        nc.sync.dma_start(out=hs_dst[:, 2:HS, :], in_=hs_src[:, 2:HS, :])
        nc.vector.tensor_copy(st_dst[:, 2:HS, :], hs_dst[:, 2:HS, :])
        nc.sync.dma_start(out=wdst[:, 5:9, :], in_=wsrc[:, 5:9, :])
        nc.vector.tensor_copy(wslab[:, 5 * COUT :], wstage[:, 5 * COUT :])

        emit_load(0, HS, 14)
        emit_load(0, 14, 21)
        emit_load(0, 21, 28)
        for i in range(1, BPC):
            for c0 in range(0, NCHUNK, 7):
                emit_load(i, c0, c0 + 7)

        if with_bias:
            bias_st = const_pool.tile([1, COUT], F32, tag="bias_st")
            nc.sync.dma_start(
                out=bias_st[:], in_=b_ap.rearrange("a b c n -> (a b c) n")
            )
            bias_sb = const_pool.tile([1, COUT], F16, tag="bias")
            nc.vector.tensor_copy(bias_sb[:], bias_st[:])
            ones_sb = const_pool.tile([1, 128], F16, tag="ones")
            nc.gpsimd.memset(ones_sb[:], 1.0)

        # PE warm-up: ~30 junk identity transposes keep the PE busy through
        # the HAM activity window (~3.4us) while the first DMAs land, so the
        # first real matmul runs at the warm 2.4 GHz clock.
        for _ in range(14):
            pwu = psum_tp.tile([CIN, CHUNK_PIX], F16, tag="pst")
            nc.tensor.transpose(
                pwu[0:CHUNK_PIX, 0:CHUNK_PIX], identity[:], identity[:]
            )

        # Per-image packed transposed slabs [cin, 1 + 58*56 rows + slop]
        xslabs = []
        for i in range(BPC):
            sl = xslab_pool.tile([CIN, SLAB_W], F16, tag=f"xs{i}")
            xslabs.append(sl)
            nc.vector.memset(sl[:, 0 : ABASE], 0.0)           # lead + top pad row
            nc.vector.memset(sl[:, (H + 1) * W + 1 : SLAB_W], 0.0)  # bottom pad + slop

        def emit_transpose(i, cidx):
            # stage [112, cin] chunk -> PE transpose -> PSUM [cin, 112] ->
            # ACT copy into the packed slab (contiguous 112 span)
            pst = psum_tp.tile([CIN, CHUNK_PIX], F16, tag="pst")
            nc.tensor.transpose(
                pst[:], stages[i][:, cidx * CIN : (cidx + 1) * CIN], identity[:]
            )
            d0 = (RPC * cidx + 1) * W + 1
            nc.scalar.activation(
                xslabs[i][:, d0 : d0 + CHUNK_PIX],
                pst[:],
                mybir.ActivationFunctionType.Copy,
            )

        # edge mini-slab: three 115-wide regions [pad a(56) sharedpad b(56)
        # pad] = A:[col0|col55] B:[col1|zeros] C:[zeros|col54], so all nine
        # edge matmuls are FULL M=113 windows (partitions 0-55 = left col,
        # 57-112 = right col; the zero halves contribute nothing) -- no
        # partition-offset PSUM writes, which measure wrong on HW.
        RA, RB, RC = 0, 115, 230
        EW = 352
        EM = 113
        ECOPY = [(RA + 1, 0), (RA + 58, 55), (RB + 1, 1), (RC + 58, 54)]

        def emit_ebuild(i):
            ed = edge_pool.tile([CIN, EW], F16, tag="E")
            nc.vector.memset(ed[:], 0.0)
            for base, col in ECOPY:
                src = (
                    xslabs[i][:, W + 1 + col : W + 1 + col + H * W]
                    .rearrange("p (r c) -> p r c", c=W)[:, :, 0:1]
                    .rearrange("p r c -> p (r c)")
                )
                nc.scalar.activation(
                    ed[:, base : base + H],
                    src,
                    mybir.ActivationFunctionType.Copy,
                )
            return ed

        def emit_window(i, w, oslab):
            q0 = ABASE + WM * w
            ps = psum_mm.tile([WM, COUT], F32, tag="psmm")
            for t in range(9):
                w0 = q0 + TAP_OFFS[t]
                nc.tensor.matmul(
                    ps[:],
                    xslabs[i][:, w0 : w0 + WM],
                    wslab[:, t * COUT : (t + 1) * COUT],
                    start=(t == 0),
                    stop=(t == 8 and not with_bias),
                )
            if with_bias:
                nc.tensor.matmul(
                    ps[:], ones_sb[:1, :WM], bias_sb[:1, :], start=False, stop=True
                )
            nc.vector.tensor_scalar_max(
                oslab[:, w * COUT : (w + 1) * COUT], ps[:], 0.0
            )

        # main-store chunk boundaries (after these windows' relu); finer at
        # the image end so the final transfer (which the edge stores
        # WAW-wait on) is small
        STORE_AT = {
            6: (0, 7),
            13: (7, 14),
            19: (14, 20),
            20: (20, 21),
            21: (21, 22),
            22: (22, 23),
            23: (23, 24),
        }

        def emit_store_chunk(i, oslab, w0, w1):
            dst = (
                y_ap[i]
                .rearrange("h w c -> (h w) c")[w0 * WM : w1 * WM, :]
                .rearrange("(w p) c -> p w c", p=WM)
            )
            src = oslab[:, w0 * COUT : w1 * COUT].rearrange(
                "p (w k) -> p w k", k=COUT
            )
            nc.gpsimd.dma_start(out=dst, in_=src)

        def emit_store_last(i, oslab):
            n = PIX - 24 * WM  # 64
            dst = y_ap[i].rearrange("h w c -> (h w) c")[24 * WM :, :]
            nc.gpsimd.dma_start(out=dst, in_=oslab[0:n, 24 * COUT : 25 * COUT])

        # edge pass: 9 full-window matmuls rebuild output cols 0 / 55
        # exactly; their stores overwrite the wrapped main-store values and
        # MUST sit behind the image's main stores on the same SWDGE queue --
        # FIFO ring order is the only cross-DMA write-ordering guarantee
        # (stores on a different queue measured racy: ~1-2 columns stale).
        E_REGION = {0: RC, 1: RA, 2: RB}  # dw -> region base

        def emit_edge(i, ed):
            pe = psum_ed.tile([EM, COUT], F32, tag="psed")
            k = 0
            nmm = 9 + (1 if with_bias else 0)
            for dh in range(3):
                for dw in range(3):
                    t = dh * 3 + dw
                    base = E_REGION[dw]
                    nc.tensor.matmul(
                        pe[:],
                        ed[:, base + dh : base + dh + EM],
                        wslab[:, t * COUT : (t + 1) * COUT],
                        start=(k == 0),
                        stop=(k == nmm - 1),
                    )
                    k += 1
            if with_bias:
                nc.tensor.matmul(
                    pe[:], ones_sb[:1, :EM], bias_sb[:1, :], start=False, stop=True
                )
            esb = edge_pool.tile([EM, COUT], F32, tag="esb")
            nc.vector.tensor_scalar_max(esb[:], pe[:], 0.0)
            return esb

        def emit_edge_stores(i, esb):
            nc.gpsimd.dma_start(out=y_ap[i][:, 0, :], in_=esb[0:H, :])
            nc.gpsimd.dma_start(out=y_ap[i][:, 55, :], in_=esb[H + 1 : H + 1 + H, :])

        # Image 0's transposes up front; image i+1's are interleaved between
        # image i's windows so the PE never waits on a bulk transpose phase.
        for c in range(NCHUNK):
            emit_transpose(0, c)
        ed = emit_ebuild(0)
        for i in range(BPC):
            oslab = out_pool.tile([WM, NW * COUT], F32, tag="osb")
            done = 0
            for w in range(NW):
                emit_window(i, w, oslab)
                if w in STORE_AT:
                    emit_store_chunk(i, oslab, *STORE_AT[w])
                if w == 21:
                    esb = emit_edge(i, ed)
                if i + 1 < BPC:
                    want = (w + 1) * NCHUNK // NW
                    while done < want:
                        emit_transpose(i + 1, done)
                        done += 1
            emit_store_last(i, oslab)
            emit_edge_stores(i, esb)
            if i + 1 < BPC:
                while done < NCHUNK:
                    emit_transpose(i + 1, done)
                    done += 1
            ed = emit_ebuild(i + 1) if i + 1 < BPC else None

    nc.compile()
    return nc


_CACHE = {}


def _get_nc(with_bias: bool):
    if with_bias not in _CACHE:
        _CACHE[with_bias] = _build(with_bias)
    return _CACHE[with_bias]


def kernel(prev_a, filter_w, filter_b):
    global LAST_RESULTS
    prev_a = np.ascontiguousarray(prev_a, dtype=np.float32)
    filter_w = np.ascontiguousarray(filter_w, dtype=np.float32)
    filter_b = np.ascontiguousarray(filter_b, dtype=np.float32).reshape(1, 1, 1, COUT)
    with_bias = bool(np.any(filter_b))
    nc = _get_nc(with_bias)
    in_maps = [
        {
            "prev_a": prev_a[c * BPC : (c + 1) * BPC],
            "filter_w": filter_w,
            "filter_b": filter_b,
        }
        for c in range(N_CORES)
    ]
    trace = os.environ.get("KERNEL_TRACE") == "1"
    res = run_bass_kernel_spmd(nc, in_maps, list(range(N_CORES)), trace=trace)
    LAST_RESULTS = res
    return np.concatenate([res.results[c]["out"] for c in range(N_CORES)], axis=0)
